# revision 8
# baseline (speedup 1.0000x reference)
"""BaggingMaxPool Trainium2 kernel.

Reference computation (N=1024, D=100000, K=20, S=256):
    for k in range(K): maxes[k] = max(inp[indices[k]], axis=0)   # [D]
    out = mean(maxes, axis=0, keepdims=True)                     # [1, D]

Strategy (8 NeuronCores, feature-dim sharding, D/8 = 12500 -> padded 12544
per core). Indices are known on the host before the kernel is built, so the
gather is compiled into static DMA access patterns - no indirect DMA needed:

  per round k (20 rounds, 256 sampled rows each):
    1. Sort the round's row indices; decompose into runs of consecutive row
       ids. Each run is one contiguous DRAM block -> one HWDGE dma_start
       into a [128, 12544] f32 SBUF tile at dest partitions = sorted rank
       (rank < 128 -> tile G_a, rank >= 128 -> tile G_b). ~200 DMAs/round.
    2. One DVE tensor_max folds G_a/G_b -> M [128, 12544] fp16 (256 rows
       pairwise-maxed down to 128 "slot" rows).
    3. TensorE transposes M in 98 [128,128] blocks (via identity matmul)
       into PSUM so the remaining reduction runs along the free axis;
       ScalarE copies PSUM -> SBUF fp16 in [128, 512] batches.
    4. DVE folds the 128 slot values per feature with 7 in-place
       tensor_max halvings (fp16 2x perf mode): T[:, :, 0:w] vs [w:2w].
    5. acc[128, 98] f32 += per-round max; after 20 rounds scale by 1/20.

Output per core is [128, 98] f32 with feature d = block*128 + partition;
the host undoes this permutation and concatenates the 8 shards.
"""

import numpy as np

import concourse.bass as bass
import concourse.tile as tile
from concourse import bacc, mybir
from concourse.bass_utils import run_bass_kernel_spmd

N = 1024
D = 100000
K = 20
S = 256
M = 8
DS = D // M          # 12500 features per core
DP = 12544           # padded to 98 * 128
C2 = DP // 128       # 98 column-blocks of 128 features
F16 = mybir.dt.float16
F32 = mybir.dt.float32


def _round_runs(idx_k: np.ndarray):
    """Sorted 256 indices -> list of (row0, length, dest_rank0); runs are
    strictly consecutive row ids, split at the dest rank-128 boundary."""
    srt = np.sort(idx_k.astype(np.int64))
    runs = []
    start = 0
    for i in range(1, S + 1):
        if i == S or srt[i] != srt[i - 1] + 1:
            runs.append((int(srt[start]), i - start, start))
            start = i
    out = []
    for row0, ln, j0 in runs:
        if j0 < 128 < j0 + ln:
            c = 128 - j0
            out.append((row0, c, j0))
            out.append((row0 + c, ln - c, 128))
        else:
            out.append((row0, ln, j0))
    return out


def build_kernel(indices: np.ndarray):
    plans = [_round_runs(indices[k]) for k in range(K)]

    nc = bacc.Bacc("TRN2", target_bir_lowering=False, debug=False, num_devices=M)
    inp = nc.dram_tensor("inp", [N, DP], F32, kind="ExternalInput")
    ident = nc.dram_tensor("ident", [128, 128], F16, kind="ExternalInput")
    out = nc.dram_tensor("out", [128, C2], F32, kind="ExternalOutput")

    with tile.TileContext(nc) as tc:
        with (
            tc.tile_pool(name="gpool", bufs=1) as gpool,
            tc.tile_pool(name="mpool", bufs=1) as mpool,
            tc.tile_pool(name="tpool", bufs=2) as tpool,
            tc.tile_pool(name="ppool", bufs=4, space="PSUM") as ppool,
            tc.tile_pool(name="rpool", bufs=1) as rpool,
        ):
            id_t = rpool.tile([128, 128], F16)
            nc.sync.dma_start(id_t[:], ident.ap())
            acc = rpool.tile([128, C2], F32)
            nc.vector.memset(acc[:], 0.0)

            ga = gpool.tile([128, DP], F32)
            gb = gpool.tile([128, DP], F32)

            for k in range(K):
                # One critical section per round: issue all row DMAs with a
                # single shared semaphore so the HWDGE rings drain freely
                # (per-DMA Tile semaphores serialize at ~1.3us each).
                gsem = nc.alloc_semaphore(f"gsem{k}")
                ndma = 0
                with tc.tile_critical():
                    r = 0
                    for row0, ln, j0 in plans[k]:
                        for i in range(ln):
                            j = j0 + i
                            dst = ga if j < 128 else gb
                            jb = j if j < 128 else j - 128
                            eng = nc.sync if (r % 2 == 0) else nc.scalar
                            # [1, DP] transfers fan out across all 16 SDMA
                            # engines via balance_dma_aps
                            eng.dma_start(
                                dst[jb:jb + 1, :],
                                inp.ap()[row0 + i:row0 + i + 1, :],
                            ).then_inc(gsem, 16)
                            ndma += 1
                            r += 1
                    nc.vector.wait_ge(gsem, ndma * 16)

                mt = mpool.tile([128, DP], F16, name=f"mt{k}", tag="mt")
                nc.vector.tensor_max(mt[:], ga[:], gb[:])

                tt = tpool.tile([128, C2, 128], F16, name=f"tt{k}", tag="tt")
                for g in range((C2 + 3) // 4):  # 25 groups of <=4 blocks
                    nblk = min(4, C2 - 4 * g)
                    ps = ppool.tile([128, 512], F16, name=f"ps{k}_{g}", tag="ps")
                    for b in range(nblk):
                        c2 = 4 * g + b
                        nc.tensor.transpose(
                            ps[:, 128 * b:128 * (b + 1)],
                            mt[:, 128 * c2:128 * (c2 + 1)],
                            id_t[:],
                        )
                    nc.scalar.copy(
                        tt[:, 4 * g:4 * g + nblk, :],
                        ps[:, 0:128 * nblk].rearrange("p (b f) -> p b f", b=nblk),
                    )

                w = 64
                while w >= 1:
                    nc.vector.tensor_max(
                        tt[:, :, 0:w], tt[:, :, 0:w], tt[:, :, w:2 * w]
                    )
                    w //= 2
                nc.vector.tensor_add(
                    acc[:], acc[:], tt[:, :, 0:1].rearrange("p c f -> p (c f)")
                )

            res = rpool.tile([128, C2], F32)
            nc.vector.tensor_scalar_mul(res[:], acc[:], 1.0 / K)
            nc.sync.dma_start(out.ap(), res[:])

    nc.compile()
    return nc


def prep_inputs(inp: np.ndarray):
    inp = np.ascontiguousarray(inp, dtype=np.float32)
    ident = np.eye(128, dtype=np.float16)
    in_maps = []
    for c in range(M):
        shard = inp[:, c * DS:(c + 1) * DS]
        shard = np.pad(shard, ((0, 0), (0, DP - DS)), mode="edge")
        in_maps.append({"inp": np.ascontiguousarray(shard), "ident": ident})
    return in_maps


def assemble_output(results) -> np.ndarray:
    parts = []
    for c in range(M):
        r = np.asarray(results[c]["out"])  # [128, 98]; d = c2*128 + p
        parts.append(r.T.reshape(-1)[:DS])
    return np.concatenate(parts)[None, :].astype(np.float32)


_NC_CACHE = {}


def kernel(inp: np.ndarray, indices: np.ndarray) -> np.ndarray:
    key = indices.tobytes()
    if _NC_CACHE.get("key") != key:
        _NC_CACHE["nc"] = build_kernel(np.asarray(indices))
        _NC_CACHE["key"] = key
    nc = _NC_CACHE["nc"]
    in_maps = prep_inputs(inp)
    res = run_bass_kernel_spmd(nc, in_maps, core_ids=list(range(M)))
    return assemble_output(res.results)


if __name__ == "__main__":
    rng = np.random.default_rng(0)
    x = rng.standard_normal((N, D), dtype=np.float32)
    ind = rng.integers(0, N, size=(K, S)).astype(np.int32)
    got = kernel(x, ind)
    exp = np.mean(
        np.stack([x[ind[k]].max(axis=0) for k in range(K)]), axis=0, keepdims=True
    )
    rel = np.abs(got - exp).max() / np.abs(exp).max()
    print("rel err:", rel)
